# revision 20
# baseline (speedup 1.0000x reference)
"""Causal self-attention Trainium2 kernel.

Problem: x:[2,2048,1024] f32, w_qkv:[1024,16,192], w_out:[16,64,1024].
  qkv = einsum('bse,ehd->bshd', x, w_qkv); q,k,v = split(qkv, 3, -1)
  att = causal_softmax(q k^T / sqrt(64)) v ;  y = einsum('bshd,hde->bse', att, w_out)

Sharding: 8 cores = batch(2) x head-group(4 heads each).  Each core computes a
partial y (its 4 heads' contribution) for its batch; the host sums the 4
partials per batch.

Per-core dataflow (all SBUF-resident between DRAM in/out, fp32r matmuls):
  xT [1024,2048] (host pre-transposed, loaded in 8 k-tile chunks)
  pass A: qkT [4x128, 2048] = w_qk^T x   (head pairs stacked: qq01,kk01,qq23,kk23)
  pass B: v_aug [2048, 4x65] = x^T w_v   (+ ones column per head -> softmax denom)
  attention per (q-512-block j, k-tile i, head-pair p), exact causal widths:
     logitsT[k,q] psum = kT_h^T qT_h ; pT = exp(0.125*logitsT) (ScalarE)
     diag tiles: multiply [128,128] tri mask on the crossing sub-block
     out_aug[65,512] psum += v_aug^T pT   (row 64 accumulates the denominator)
  normalize: recip(denom row) -> ones-matmul partition broadcast -> DVE mul
  out-proj: y[s,e] psum = attnT^T w_o ; 1MB stores.
"""

import numpy as np

B, S, E = 2, 2048, 1024
H, D = 16, 64
HC = 4          # heads per core
NCORES = 8
SB = S // 512   # 4 q-blocks of 512
KT = S // 128   # 16 k-tiles of 128
ET = E // 128   # 8 e-tiles

_cached = {}


def _build_program(unroll=1):
    import concourse.bass as bass  # noqa: F401
    import concourse.tile as tile
    from concourse import bacc, mybir
    from contextlib import ExitStack

    f32 = mybir.dt.float32
    f32r = mybir.dt.float32r
    Exp = mybir.ActivationFunctionType.Exp

    nc = bacc.Bacc("TRN2", target_bir_lowering=False, debug=False)
    xT_d = nc.declare_dram_parameter("xT", [E, S], f32, isOutput=False)
    wqk_d = nc.declare_dram_parameter("w_qk", [E, 512], f32, isOutput=False)
    wv_d = nc.declare_dram_parameter("w_v", [E, 256], f32, isOutput=False)
    wo_d = nc.declare_dram_parameter("w_o", [256, E], f32, isOutput=False)
    tri_d = nc.declare_dram_parameter("tri", [128, 128], f32, isOutput=False)
    y_d = nc.declare_dram_parameter("y", [S, E], f32, isOutput=True)

    xT_r = xT_d.rearrange("(t p) s -> p t s", p=128).bitcast(f32r)

    with tile.TileContext(nc) as tc:
      for _rep in range(unroll):
        with ExitStack() as ctx:
            persist = ctx.enter_context(tc.tile_pool(name="persist", bufs=1))

            # ---- persistent SBUF buffers (f32r: consumed by matmuls) ----
            xT_sb = persist.tile([128, ET, S], f32r, tag="xT")
            wqk_sb = persist.tile([128, ET, 512], f32r, tag="wqk")
            wv_sb = persist.tile([128, ET, 256], f32r, tag="wv")
            wo_sb = persist.tile([128, 2, E], f32r, tag="wo")
            tri_sb = persist.tile([128, 128], f32r, tag="tri")
            ones_sb = persist.tile([128, 64], f32r, tag="ones")
            qk_sb = persist.tile([128, 4, S], f32r, tag="qk")
            v_sb = persist.tile([128, KT, HC, 65], f32r, tag="v")
            at_sb = persist.tile([128, 2, S], f32r, tag="attnT")

            nc.sync.dma_start(
                out=wqk_sb, in_=wqk_d.rearrange("(t p) m -> p t m", p=128).bitcast(f32r))
            nc.sync.dma_start(
                out=wv_sb, in_=wv_d.rearrange("(t p) m -> p t m", p=128).bitcast(f32r))
            nc.sync.dma_start(
                out=wo_sb, in_=wo_d.rearrange("(t p) m -> p t m", p=128).bitcast(f32r))
            nc.sync.dma_start(out=tri_sb, in_=tri_d[:, :].bitcast(f32r))
            for k in range(ET):  # chunked so pass A can start early
                nc.sync.dma_start(out=xT_sb[:, k, :], in_=xT_r[:, k, :])
            nc.vector.memset(v_sb[:, :, :, 64:65].bitcast(f32), 1.0)
            nc.vector.memset(ones_sb.bitcast(f32), 1.0)

            # ---- pass A: qkT = w_qk^T @ xT  -> qk_sb[:, m, :] ----
            with tc.tile_pool(name="psA", bufs=4, space="PSUM") as psA:
                for m in range(4):
                    for n in range(SB):
                        ps = psA.tile([128, 512], f32, tag="psA")
                        for k in range(ET):
                            nc.tensor.matmul(
                                ps,
                                wqk_sb[:, k, m * 128:(m + 1) * 128],
                                xT_sb[:, k, n * 512:(n + 1) * 512],
                                start=(k == 0),
                                stop=(k == ET - 1),
                            )
                        nc.vector.tensor_copy(qk_sb[:, m, n * 512:(n + 1) * 512], ps)

            # ---- pass B: v = x @ w_v -> v_sb[:, t, h, 0:64] ----
            with tc.tile_pool(name="psB", bufs=4, space="PSUM") as psB:
                for t in range(KT):
                    ps = psB.tile([128, 256], f32, tag="psB")
                    for k in range(ET):
                        nc.tensor.matmul(
                            ps,
                            xT_sb[:, k, t * 128:(t + 1) * 128],
                            wv_sb[:, k, :],
                            start=(k == 0),
                            stop=(k == ET - 1),
                        )
                    nc.vector.tensor_copy(
                        v_sb[:, t, :, 0:64],
                        ps.rearrange("p (h d) -> p h d", h=HC),
                    )

            # ---- attention ----
            with tc.tile_pool(name="psL", bufs=2, space="PSUM") as psL, \
                 tc.tile_pool(name="psO", bufs=4, space="PSUM") as psO, \
                 tc.tile_pool(name="pt", bufs=3) as ptp, \
                 tc.tile_pool(name="norm", bufs=2) as normp:
                for j in range(SB):
                    oa = [psO.tile([65, 512], f32, tag="oa", name=f"oa_{j}_{h}")
                          for h in range(HC)]
                    nk = 4 * j + 4  # k-tiles for this q-block
                    # ascending i: PV for i=0 is always full width (512) and
                    # carries start=True; diagonal k-tiles accumulate into the
                    # already-written [off:512] sub-range only.
                    for i in range(nk):
                        off = max(0, (i - 4 * j) * 128)  # first valid q col
                        q0 = j * 512 + off
                        for p in range(2):  # head pairs (0,1), (2,3)
                            lg = psL.tile([128, 2, 512], f32, tag="lg")
                            for sub in range(2):
                                r0 = sub * 64
                                nc.tensor.matmul(
                                    lg[:, sub, off:512],
                                    qk_sb[r0:r0 + 64, 2 * p + 1, i * 128:(i + 1) * 128],
                                    qk_sb[r0:r0 + 64, 2 * p, q0:(j + 1) * 512],
                                    start=True,
                                    stop=True,
                                )
                            pt = ptp.tile([128, 2, 512], f32r, tag="pt")
                            nc.scalar.activation(
                                pt[:, :, off:512], lg[:, :, off:512],
                                Exp, scale=0.125)
                            if i >= 4 * j:  # diagonal-crossing k-tile
                                for sub in range(2):
                                    nc.vector.tensor_mul(
                                        pt[:, sub, off:off + 128],
                                        pt[:, sub, off:off + 128],
                                        tri_sb,
                                    )
                            for sub in range(2):
                                h = 2 * p + sub
                                nc.tensor.matmul(
                                    oa[h][:, off:512],
                                    v_sb[:, i, h, 0:65],
                                    pt[:, sub, off:512],
                                    start=(i == 0),
                                    stop=(i == nk - 1),
                                    skip_group_check=True,
                                )
                    # normalize: attnT[h rows, j block] = oa[h][0:64] / oa[h][64]
                    for h in range(HC):
                        tmp = normp.tile([65, 512], f32, tag="tmp")
                        rr = normp.tile([65, 512], f32r, tag="rr")
                        nc.scalar.copy(tmp, oa[h])
                        nc.vector.reciprocal(tmp[64:65, :], tmp[64:65, :])
                        nc.vector.tensor_copy(rr[64:65, :], tmp[64:65, :])
                        # broadcast recip row across 64 partitions via K=1 matmul
                        # (lhsT/rhs both at base partition 64 -> row group 64);
                        # lives in a logits slot so oa slots free up for j+1
                        bc = psL.tile([64, 512], f32, tag="lg", name=f"bc_{j}_{h}")
                        nc.tensor.matmul(
                            bc, ones_sb[64:65, :], rr[64:65, :],
                            start=True, stop=True)
                        if h % 2 == 0:
                            nc.vector.tensor_mul(
                                at_sb[0:64, h // 2, j * 512:(j + 1) * 512],
                                tmp[0:64, :], bc)
                        else:
                            tn = normp.tile([64, 512], f32r, tag="tn")
                            nc.vector.tensor_mul(tn, tmp[0:64, :], bc)
                            nc.sync.dma_start(
                                out=at_sb[64:128, h // 2, j * 512:(j + 1) * 512],
                                in_=tn)

            # ---- output projection: y = attnT^T @ w_o ----
            with tc.tile_pool(name="psY", bufs=4, space="PSUM") as psY, \
                 tc.tile_pool(name="ysb", bufs=2) as ysb:
                for th in range(KT // 2):  # pairs of s-tiles -> 1MB stores
                    yt = ysb.tile([128, 2, 1024], f32, tag="yt")
                    for tt in range(2):
                        t = 2 * th + tt
                        for n in range(2):
                            ps = psY.tile([128, 512], f32, tag="psY")
                            for c in range(2):
                                nc.tensor.matmul(
                                    ps,
                                    at_sb[:, c, t * 128:(t + 1) * 128],
                                    wo_sb[:, c, n * 512:(n + 1) * 512],
                                    start=(c == 0),
                                    stop=(c == 1),
                                )
                            nc.vector.tensor_copy(
                                yt[:, tt, n * 512:(n + 1) * 512], ps)
                    nc.sync.dma_start(
                        out=y_d[th * 256:(th + 1) * 256, :].rearrange(
                            "(t p) e -> p t e", p=128),
                        in_=yt)
    nc.compile()
    return nc


def _prep_inputs(x, w_qkv, w_out):
    """Build the 8 per-core input maps. Core c = batch(c//4), head-group(c%4)."""
    tri = np.triu(np.ones((128, 128), dtype=np.float32))  # [k,q] keep k<=q
    xT = [np.ascontiguousarray(x[b].T).astype(np.float32) for b in range(B)]
    in_maps = []
    for c in range(NCORES):
        b, g = c // 4, c % 4
        hs = [g * HC + l for l in range(HC)]
        # w_qk [1024, 512]: m-tiles = [q_h0|q_h1], [k_h0|k_h1], [q_h2|q_h3], [k_h2|k_h3]
        cols = []
        for pair in range(2):
            h0, h1 = hs[2 * pair], hs[2 * pair + 1]
            cols.append(np.concatenate([w_qkv[:, h0, 0:64], w_qkv[:, h1, 0:64]], axis=1))
            cols.append(np.concatenate([w_qkv[:, h0, 64:128], w_qkv[:, h1, 64:128]], axis=1))
        w_qk = np.ascontiguousarray(np.concatenate(cols, axis=1), dtype=np.float32)
        w_v = np.ascontiguousarray(
            np.concatenate([w_qkv[:, h, 128:192] for h in hs], axis=1), dtype=np.float32)
        w_o = np.ascontiguousarray(
            w_out[hs[0]:hs[0] + HC].reshape(HC * D, E), dtype=np.float32)
        in_maps.append({"xT": xT[b], "w_qk": w_qk, "w_v": w_v, "w_o": w_o, "tri": tri})
    return in_maps


def kernel(x, w_qkv, w_out):
    from concourse.bass_utils import run_bass_kernel_spmd

    if "nc" not in _cached:
        _cached["nc"] = _build_program()
    nc = _cached["nc"]
    in_maps = _prep_inputs(np.asarray(x), np.asarray(w_qkv), np.asarray(w_out))
    res = run_bass_kernel_spmd(nc, in_maps, list(range(NCORES))).results
    y = np.zeros((B, S, E), dtype=np.float32)
    for c in range(NCORES):
        y[c // 4] += res[c]["y"]
    return y


# revision 22
# speedup vs baseline: 3.2339x; 3.2339x over previous
"""Causal self-attention Trainium2 kernel.

Problem: x:[2,2048,1024] f32, w_qkv:[1024,16,192], w_out:[16,64,1024].
  qkv = einsum('bse,ehd->bshd', x, w_qkv); q,k,v = split(qkv, 3, -1)
  att = causal_softmax(q k^T / sqrt(64)) v ;  y = einsum('bshd,hde->bse', att, w_out)

Sharding: 8 cores = batch(2) x head-group(4 heads each).  Each core computes a
partial y (its 4 heads' contribution) for its batch; the host sums the 4
partials per batch.

Per-core dataflow (all SBUF-resident between DRAM in/out, fp32r matmuls):
  xT [1024,2048] (host pre-transposed, loaded in 8 k-tile chunks)
  pass A: qkT [4x128, 2048] = w_qk^T x   (head pairs stacked: qq01,kk01,qq23,kk23)
  pass B: v_aug [2048, 4x65] = x^T w_v   (+ ones column per head -> softmax denom)
  attention per (q-512-block j, k-tile i, head-pair p), exact causal widths:
     logitsT[k,q] psum = kT_h^T qT_h ; pT = exp(0.125*logitsT) (ScalarE)
     diag tiles: multiply [128,128] tri mask on the crossing sub-block
     out_aug[65,512] psum += v_aug^T pT   (row 64 accumulates the denominator)
  normalize: recip(denom row) -> ones-matmul partition broadcast -> DVE mul
  out-proj: y[s,e] psum = attnT^T w_o ; 1MB stores.
"""

import numpy as np

B, S, E = 2, 2048, 1024
H, D = 16, 64
HC = 4          # heads per core
NCORES = 8
SB = S // 512   # 4 q-blocks of 512
KT = S // 128   # 16 k-tiles of 128
ET = E // 128   # 8 e-tiles

_cached = {}


def _build_program(unroll=1):
    import concourse.bass as bass  # noqa: F401
    import concourse.tile as tile
    from concourse import bacc, mybir
    from contextlib import ExitStack

    f32 = mybir.dt.float32
    f32r = mybir.dt.float32r
    Exp = mybir.ActivationFunctionType.Exp

    nc = bacc.Bacc("TRN2", target_bir_lowering=False, debug=False)
    xT_d = nc.declare_dram_parameter("xT", [E, S], f32, isOutput=False)
    wqk_d = nc.declare_dram_parameter("w_qk", [E, 512], f32, isOutput=False)
    wv_d = nc.declare_dram_parameter("w_v", [E, 256], f32, isOutput=False)
    wo_d = nc.declare_dram_parameter("w_o", [256, E], f32, isOutput=False)
    tri_d = nc.declare_dram_parameter("tri", [128, 128], f32, isOutput=False)
    y_d = nc.declare_dram_parameter("y", [S, E], f32, isOutput=True)

    xT_r = xT_d.rearrange("(t p) s -> p t s", p=128).bitcast(f32r)

    with tile.TileContext(nc) as tc:
      for _rep in range(unroll):
        with ExitStack() as ctx:
            persist = ctx.enter_context(tc.tile_pool(name="persist", bufs=1))

            # ---- persistent SBUF buffers (f32r: consumed by matmuls) ----
            xT_sb = persist.tile([128, ET, S], f32r, tag="xT")
            wqk_sb = persist.tile([128, ET, 512], f32r, tag="wqk")
            wv_sb = persist.tile([128, ET, 256], f32r, tag="wv")
            wo_sb = persist.tile([128, 2, E], f32r, tag="wo")
            tri_sb = persist.tile([128, 128], f32r, tag="tri")
            ones_sb = persist.tile([128, 64], f32r, tag="ones")
            qk_sb = persist.tile([128, 4, S], f32r, tag="qk")
            v_sb = persist.tile([128, KT, HC, 65], f32r, tag="v")
            at_sb = persist.tile([128, 2, S], f32r, tag="attnT")

            nc.sync.dma_start(
                out=wqk_sb, in_=wqk_d.rearrange("(t p) m -> p t m", p=128).bitcast(f32r))
            nc.sync.dma_start(
                out=wv_sb, in_=wv_d.rearrange("(t p) m -> p t m", p=128).bitcast(f32r))
            nc.sync.dma_start(
                out=wo_sb, in_=wo_d.rearrange("(t p) m -> p t m", p=128).bitcast(f32r))
            nc.sync.dma_start(out=tri_sb, in_=tri_d[:, :].bitcast(f32r))
            for k in range(ET):  # chunked so pass A can start early
                nc.sync.dma_start(out=xT_sb[:, k, :], in_=xT_r[:, k, :])
            nc.vector.memset(v_sb[:, :, :, 64:65].bitcast(f32), 1.0)
            nc.vector.memset(ones_sb.bitcast(f32), 1.0)

            # ---- pass A: qkT = w_qk^T @ xT  -> qk_sb[:, m, :] ----
            with tc.tile_pool(name="psA", bufs=4, space="PSUM") as psA:
                for m in range(4):
                    # k-outer / n-inner: the stationary w tile stays loaded
                    # across the 4 n matmuls at each k
                    ps = [psA.tile([128, 512], f32, tag="psA", name=f"psA_{m}_{n}")
                          for n in range(SB)]
                    for k in range(ET):
                        for n in range(SB):
                            nc.tensor.matmul(
                                ps[n],
                                wqk_sb[:, k, m * 128:(m + 1) * 128],
                                xT_sb[:, k, n * 512:(n + 1) * 512],
                                start=(k == 0),
                                stop=(k == ET - 1),
                            )
                    for n in range(SB):
                        nc.vector.tensor_copy(
                            qk_sb[:, m, n * 512:(n + 1) * 512], ps[n])

            # ---- pass B: v = x @ w_v -> v_sb[:, t, h, 0:64] ----
            with tc.tile_pool(name="psB", bufs=4, space="PSUM") as psB:
                for t in range(KT):
                    ps = psB.tile([128, 256], f32, tag="psB")
                    for k in range(ET):
                        nc.tensor.matmul(
                            ps,
                            xT_sb[:, k, t * 128:(t + 1) * 128],
                            wv_sb[:, k, :],
                            start=(k == 0),
                            stop=(k == ET - 1),
                        )
                    nc.vector.tensor_copy(
                        v_sb[:, t, :, 0:64],
                        ps.rearrange("p (h d) -> p h d", h=HC),
                    )

            # ---- attention ----
            with tc.tile_pool(name="psL", bufs=2, space="PSUM") as psL, \
                 tc.tile_pool(name="psO", bufs=4, space="PSUM") as psO, \
                 tc.tile_pool(name="pt", bufs=3) as ptp, \
                 tc.tile_pool(name="norm", bufs=2) as normp:
                for j in range(SB):
                    oa = [psO.tile([65, 512], f32, tag="oa", name=f"oa_{j}_{h}")
                          for h in range(HC)]
                    nk = 4 * j + 4  # k-tiles for this q-block
                    # ascending i: PV for i=0 is always full width (512) and
                    # carries start=True; diagonal k-tiles accumulate into the
                    # already-written [off:512] sub-range only.
                    for i in range(nk):
                        off = max(0, (i - 4 * j) * 128)  # first valid q col
                        q0 = j * 512 + off
                        for p in range(2):  # head pairs (0,1), (2,3)
                            lg = psL.tile([128, 2, 512], f32, tag="lg")
                            for sub in range(2):
                                r0 = sub * 64
                                nc.tensor.matmul(
                                    lg[:, sub, off:512],
                                    qk_sb[r0:r0 + 64, 2 * p + 1, i * 128:(i + 1) * 128],
                                    qk_sb[r0:r0 + 64, 2 * p, q0:(j + 1) * 512],
                                    start=True,
                                    stop=True,
                                )
                            pt = ptp.tile([128, 2, 512], f32r, tag="pt")
                            nc.scalar.activation(
                                pt[:, :, off:512], lg[:, :, off:512],
                                Exp, scale=0.125)
                            if i >= 4 * j:  # diagonal-crossing k-tile
                                for sub in range(2):
                                    nc.vector.tensor_mul(
                                        pt[:, sub, off:off + 128],
                                        pt[:, sub, off:off + 128],
                                        tri_sb,
                                    )
                            for sub in range(2):
                                h = 2 * p + sub
                                nc.tensor.matmul(
                                    oa[h][:, off:512],
                                    v_sb[:, i, h, 0:65],
                                    pt[:, sub, off:512],
                                    start=(i == 0),
                                    stop=(i == nk - 1),
                                    skip_group_check=True,
                                )
                    # normalize: attnT[h rows, j block] = oa[h][0:64] / oa[h][64]
                    for h in range(HC):
                        tmp = normp.tile([65, 512], f32, tag="tmp")
                        rr = normp.tile([65, 512], f32r, tag="rr")
                        nc.scalar.copy(tmp, oa[h])
                        nc.vector.reciprocal(tmp[64:65, :], tmp[64:65, :])
                        nc.vector.tensor_copy(rr[64:65, :], tmp[64:65, :])
                        # broadcast recip row across 64 partitions via K=1 matmul
                        # (lhsT/rhs both at base partition 64 -> row group 64);
                        # lives in a logits slot so oa slots free up for j+1
                        bc = psL.tile([64, 512], f32, tag="lg", name=f"bc_{j}_{h}")
                        nc.tensor.matmul(
                            bc, ones_sb[64:65, :], rr[64:65, :],
                            start=True, stop=True)
                        if h % 2 == 0:
                            nc.vector.tensor_mul(
                                at_sb[0:64, h // 2, j * 512:(j + 1) * 512],
                                tmp[0:64, :], bc)
                        else:
                            tn = normp.tile([64, 512], f32r, tag="tn")
                            nc.vector.tensor_mul(tn, tmp[0:64, :], bc)
                            nc.sync.dma_start(
                                out=at_sb[64:128, h // 2, j * 512:(j + 1) * 512],
                                in_=tn)

            # ---- output projection: y = attnT^T @ w_o ----
            with tc.tile_pool(name="psY", bufs=4, space="PSUM") as psY, \
                 tc.tile_pool(name="ysb", bufs=2) as ysb:
                for th in range(KT // 2):  # pairs of s-tiles -> 1MB stores
                    yt = ysb.tile([128, 2, 1024], f32, tag="yt")
                    for tt in range(2):
                        t = 2 * th + tt
                        ps = [psY.tile([128, 512], f32, tag="psY",
                                       name=f"psY_{t}_{n}") for n in range(2)]
                        for c in range(2):  # stationary at-tile reused over n
                            for n in range(2):
                                nc.tensor.matmul(
                                    ps[n],
                                    at_sb[:, c, t * 128:(t + 1) * 128],
                                    wo_sb[:, c, n * 512:(n + 1) * 512],
                                    start=(c == 0),
                                    stop=(c == 1),
                                )
                        for n in range(2):
                            nc.vector.tensor_copy(
                                yt[:, tt, n * 512:(n + 1) * 512], ps[n])
                    nc.sync.dma_start(
                        out=y_d[th * 256:(th + 1) * 256, :].rearrange(
                            "(t p) e -> p t e", p=128),
                        in_=yt)
    nc.compile()
    return nc


def _prep_inputs(x, w_qkv, w_out):
    """Build the 8 per-core input maps. Core c = batch(c//4), head-group(c%4)."""
    tri = np.triu(np.ones((128, 128), dtype=np.float32))  # [k,q] keep k<=q
    xT = [np.ascontiguousarray(x[b].T).astype(np.float32) for b in range(B)]
    in_maps = []
    for c in range(NCORES):
        b, g = c // 4, c % 4
        hs = [g * HC + l for l in range(HC)]
        # w_qk [1024, 512]: m-tiles = [q_h0|q_h1], [k_h0|k_h1], [q_h2|q_h3], [k_h2|k_h3]
        cols = []
        for pair in range(2):
            h0, h1 = hs[2 * pair], hs[2 * pair + 1]
            cols.append(np.concatenate([w_qkv[:, h0, 0:64], w_qkv[:, h1, 0:64]], axis=1))
            cols.append(np.concatenate([w_qkv[:, h0, 64:128], w_qkv[:, h1, 64:128]], axis=1))
        w_qk = np.ascontiguousarray(np.concatenate(cols, axis=1), dtype=np.float32)
        w_v = np.ascontiguousarray(
            np.concatenate([w_qkv[:, h, 128:192] for h in hs], axis=1), dtype=np.float32)
        w_o = np.ascontiguousarray(
            w_out[hs[0]:hs[0] + HC].reshape(HC * D, E), dtype=np.float32)
        in_maps.append({"xT": xT[b], "w_qk": w_qk, "w_v": w_v, "w_o": w_o, "tri": tri})
    return in_maps


def kernel(x, w_qkv, w_out):
    from concourse.bass_utils import run_bass_kernel_spmd

    if "nc" not in _cached:
        _cached["nc"] = _build_program()
    nc = _cached["nc"]
    in_maps = _prep_inputs(np.asarray(x), np.asarray(w_qkv), np.asarray(w_out))
    res = run_bass_kernel_spmd(nc, in_maps, list(range(NCORES))).results
    y = np.zeros((B, S, E), dtype=np.float32)
    for c in range(NCORES):
        y[c // 4] += res[c]["y"]
    return y
